# revision 2
# baseline (speedup 1.0000x reference)
"""CrossNetwork (DCN) forward on 8 TRN2 NeuronCores.

Reference computation (per cross layer i, x0 = input):
    s_i = xl . w_i            (per-row scalar)
    xl  = x0 * s_i + b_i + xl

Algebraic collapse: xl_i = alpha_i * x0 + c_i with per-row scalar alpha_i
and a row-constant vector c_i = sum_{j<i} b_j. Hence:
    u_i       = x0 . w_i                      (3 dots per row, all vs x0)
    alpha_0   = 1,  alpha_{i+1} = alpha_i * (1 + u_i) + (c_i . w_i)
    out       = alpha_3 * x0 + c_3
One read of x, one write of out -> memory roofline (32 MiB HBM per core).

Sharding: pure data parallel over the batch dim, weights replicated.

zero-b fast path (the reference always passes b = 0): the three dots are
computed on the otherwise-idle PE array.  Each 128-row sub-tile is
transposed on-chip (PE transpose -> PSUM -> DVE/ACT copy to SBUF), then a
second accumulating matmul contracts over D with the host-packed W^T as
the moving operand, producing u = [128 rows, 4] directly in PSUM with
rows on partitions.  alpha3 = (1+u0)(1+u1)(1+u2) is three tiny ACT ops;
the final scale out = alpha3 * x is split DVE/ACT.  fp32 end to end.
Elementwise engines only do: PSUM->SBUF copy (1 pass) + scale (1 pass),
so every engine stays well under the ~94 us DMA roofline, with DMA in
2 MiB transfers (loads on SP HWDGE, stores on Pool SWDGE).

A general-b variant (full constants) is kept for the b != 0 case.
"""

import contextlib

import numpy as np

import concourse.bacc as bacc
import concourse.mybir as mybir
import concourse.tile as tile
from concourse.bass_utils import run_bass_kernel_spmd

N_CORES = 8
B, D, CROSS = 16384, 2048, 3
P = 128
NB = D // P  # 16 column blocks per sub-tile
F32 = mybir.dt.float32


def build_body_zero_b(tc, x_ap, wt_ap, ident_ap, out_ap, rows, reps=1):
    """b == 0 specialization: out = alpha3 * x, dots on the PE array.

    x_ap/out_ap are [rows//2, 2*D] views of the row-major [rows, D] data:
    big tile t = 128 view-rows = 256 data rows; sub-tile j in {0,1} is
    x[:, j*D:(j+1)*D] = data rows 256*t + 2*p + j.

    reps > 1 repeats the main loop in-NEFF (benchmarking only).
    """
    nc = tc.nc
    nt = rows // 256  # big tiles per pass
    Act = mybir.ActivationFunctionType

    with contextlib.ExitStack() as ctx:
        const = ctx.enter_context(tc.tile_pool(name="const", bufs=1))
        xpool = ctx.enter_context(tc.tile_pool(name="x", bufs=4))
        ypool = ctx.enter_context(tc.tile_pool(name="y", bufs=3))
        xtpool = ctx.enter_context(tc.tile_pool(name="xT", bufs=3))
        ptpool = ctx.enter_context(
            tc.tile_pool(name="pT", bufs=6, space="PSUM")
        )
        utpool = ctx.enter_context(
            tc.tile_pool(name="u", bufs=2, space="PSUM")
        )
        apool = ctx.enter_context(tc.tile_pool(name="a", bufs=12))

        ident = const.tile([P, P], F32, tag="I")
        nc.scalar.dma_start(out=ident[:], in_=ident_ap[:, :])
        wt = const.tile([P, 4 * NB], F32, tag="Wt")
        nc.scalar.dma_start(out=wt[:], in_=wt_ap[:, :])

        def stage_b(st):
            """Dots + recurrence + scale + (on j==1) store for one sub-tile."""
            xs, xts, yt, t, j, k = st
            ut = utpool.tile([P, 4], F32, tag="u")
            for b in range(NB):
                nc.tensor.matmul(
                    out=ut[:],
                    lhsT=xts[:, b * P : (b + 1) * P],
                    rhs=wt[:, b * 4 : (b + 1) * 4],
                    start=(b == 0),
                    stop=(b == NB - 1),
                )
            # alpha3 = (1+u0)(1+u1)(1+u2): ACT tiny ops on [P, 1]
            t1 = apool.tile([P, 1], F32, tag="t1")
            nc.scalar.add(t1[:], ut[:, 0:1], 1.0)
            a2 = apool.tile([P, 1], F32, tag="a2")
            nc.scalar.activation(a2[:], ut[:, 1:2], Act.Identity,
                                 bias=t1[:], scale=t1[:])
            a3 = apool.tile([P, 1], F32, tag="a3")
            nc.scalar.activation(a3[:], ut[:, 2:3], Act.Identity,
                                 bias=a2[:], scale=a2[:])
            # out = alpha3 * x0
            ys = yt[:, j * D : (j + 1) * D]
            if k % 2 == 0:
                nc.vector.tensor_scalar_mul(ys, xs, a3[:])
            else:
                nc.scalar.activation(ys, xs, Act.Copy, scale=a3[:])
            if j == 1:
                t_ = t % nt
                nc.gpsimd.dma_start(
                    out=out_ap[t_ * P : (t_ + 1) * P, :], in_=yt[:]
                )

        prev = None
        xt = yt = None
        for k in range(nt * reps * 2):
            t, j = k // 2, k % 2
            if j == 0:
                t_ = t % nt
                xt = xpool.tile([P, 2 * D], F32, tag="x")
                nc.sync.dma_start(
                    out=xt[:], in_=x_ap[t_ * P : (t_ + 1) * P, :]
                )
                yt = ypool.tile([P, 2 * D], F32, tag="y")
            xs = xt[:, j * D : (j + 1) * D]

            # stage A: 16 PE transposes into 4 PSUM bank tiles, then 4
            # PSUM->SBUF copies (DVE/ACT split)
            pts = []
            for c in range(4):
                pt = ptpool.tile([P, 512], F32, tag="pT")
                for q in range(4):
                    b = c * 4 + q
                    nc.tensor.transpose(
                        pt[:, q * P : (q + 1) * P],
                        xs[:, b * P : (b + 1) * P],
                        ident[:],
                    )
                pts.append(pt)
            xts = xtpool.tile([P, D], F32, tag="xT")
            for c in range(4):
                dst = xts[:, c * 512 : (c + 1) * 512]
                if c < 2:
                    nc.vector.tensor_copy(dst, pts[c][:])
                else:
                    nc.scalar.activation(dst, pts[c][:], Act.Copy)

            # stage B of the previous sub-tile (keeps PE pipelined: its
            # u-matmuls wait on copies that overlap this sub-tile's
            # transposes)
            if prev is not None:
                stage_b(prev)
            prev = (xs, xts, yt, t, j, k)
        stage_b(prev)


def build_body_general(tc, x_ap, w_ap, b_ap, out_ap, rows):
    """General-b path: full constants, final = ACT scale + Pool bias-add."""
    nc = tc.nc
    nt = rows // P
    Al = mybir.AluOpType
    Act = mybir.ActivationFunctionType

    with contextlib.ExitStack() as ctx:
        const = ctx.enter_context(tc.tile_pool(name="const", bufs=1))
        xpool = ctx.enter_context(tc.tile_pool(name="x", bufs=4))
        ypool = ctx.enter_context(tc.tile_pool(name="y", bufs=4))
        spool = ctx.enter_context(tc.tile_pool(name="scr", bufs=3))
        upool = ctx.enter_context(tc.tile_pool(name="u", bufs=16))

        # Load each tiny w_i / b_i row to partition 0, then replicate across
        # all 128 partitions on-chip (gpsimd partition_broadcast). The custom
        # op requires its input AP to start at partition 0, hence one [1, D]
        # tile per row. All row tiles are transient (pre pool).
        with tc.tile_pool(name="pre", bufs=1) as pre:
            wrow = []
            brow = []
            for i in range(CROSS):
                wr = pre.tile([1, D], F32, tag=f"wr{i}")
                nc.sync.dma_start(out=wr[:], in_=w_ap[i : i + 1, :])
                wrow.append(wr)
                br = pre.tile([1, D], F32, tag=f"br{i}")
                nc.sync.dma_start(out=br[:], in_=b_ap[i : i + 1, :])
                brow.append(br)

            wbc = []
            for i in range(CROSS):
                wt = const.tile([P, D], F32, tag=f"w{i}")
                nc.gpsimd.partition_broadcast(wt[:], wrow[i][:])
                wbc.append(wt)

            # row constants on [1, D]: c2 = b0 + b1, c3 = c2 + b2
            c2row = pre.tile([1, D], F32, tag="c2r")
            nc.vector.tensor_add(c2row[:], brow[0][:], brow[1][:])
            c3row = pre.tile([1, D], F32, tag="c3r")
            nc.vector.tensor_add(c3row[:], c2row[:], brow[2][:])
            c3bc = const.tile([P, D], F32, tag="c3")
            nc.gpsimd.partition_broadcast(c3bc[:], c3row[:])

            # k1 = b0 . w1, k2 = c2 . w2 (scalars), then replicate to [P, 1]
            k1row = pre.tile([1, 1], F32, tag="k1r")
            scr_k1 = pre.tile([1, D], F32, tag="scrr")
            nc.vector.scalar_tensor_tensor(
                out=scr_k1[:], in0=brow[0][:], scalar=0.0, in1=wrow[1][:],
                op0=Al.bypass, op1=Al.mult, accum_out=k1row[:],
            )
            k2row = pre.tile([1, 1], F32, tag="k2r")
            scr_k2 = pre.tile([1, D], F32, tag="scrr2")
            nc.vector.scalar_tensor_tensor(
                out=scr_k2[:], in0=c2row[:], scalar=0.0, in1=wrow[2][:],
                op0=Al.bypass, op1=Al.mult, accum_out=k2row[:],
            )
            k1bc = const.tile([P, 1], F32, tag="k1")
            nc.gpsimd.partition_broadcast(k1bc[:], k1row[:])
            k2bc = const.tile([P, 1], F32, tag="k2")
            nc.gpsimd.partition_broadcast(k2bc[:], k2row[:])

        for t in range(nt):
            xt = xpool.tile([P, D], F32, tag="x")
            nc.sync.dma_start(out=xt[:], in_=x_ap[t * P : (t + 1) * P, :])

            us = []
            for i in range(CROSS):
                u = upool.tile([P, 1], F32, tag=f"u{i}")
                scr = spool.tile([P, D], F32, tag="scr")
                nc.vector.scalar_tensor_tensor(
                    out=scr[:], in0=xt[:], scalar=0.0, in1=wbc[i][:],
                    op0=Al.bypass, op1=Al.mult, accum_out=u[:],
                )
                us.append(u)

            # alpha recurrence on ACT: a3 = ((1+u0)(1+u1) + k1)(1+u2) + k2
            t1 = upool.tile([P, 1], F32, tag="t1")
            nc.scalar.add(t1[:], us[0][:], 1.0)
            t2 = upool.tile([P, 1], F32, tag="t2")
            nc.scalar.add(t2[:], us[1][:], 1.0)
            a2 = upool.tile([P, 1], F32, tag="a2")
            nc.scalar.activation(a2[:], t2[:], Act.Identity, bias=k1bc[:], scale=t1[:])
            t3 = upool.tile([P, 1], F32, tag="t3")
            nc.scalar.add(t3[:], us[2][:], 1.0)
            a3 = upool.tile([P, 1], F32, tag="a3")
            nc.scalar.activation(a3[:], t3[:], Act.Identity, bias=k2bc[:], scale=a2[:])

            # out = alpha3 * x0 + c3: scale on ACT, bias-add in place on Pool
            yt = ypool.tile([P, D], F32, tag="y")
            nc.scalar.activation(yt[:], xt[:], Act.Copy, scale=a3[:])
            nc.gpsimd.tensor_tensor(out=yt[:], in0=yt[:], in1=c3bc[:], op=Al.add)
            nc.sync.dma_start(out=out_ap[t * P : (t + 1) * P, :], in_=yt[:])


_CACHE = {}


def get_nc(rows, zero_b=False, reps=1):
    key = (rows, zero_b, reps)
    if key not in _CACHE:
        nc = bacc.Bacc(
            "TRN2",
            target_bir_lowering=False,
            debug=False,
            enable_asserts=False,
            num_devices=N_CORES,
        )
        if zero_b:
            x = nc.dram_tensor("x", [rows // 2, 2 * D], F32,
                               kind="ExternalInput").ap()
            wt = nc.dram_tensor("Wt", [P, 4 * NB], F32,
                                kind="ExternalInput").ap()
            ident = nc.dram_tensor("I", [P, P], F32, kind="ExternalInput").ap()
            out = nc.dram_tensor("out", [rows // 2, 2 * D], F32,
                                 kind="ExternalOutput").ap()
            with tile.TileContext(nc) as tc:
                build_body_zero_b(tc, x, wt, ident, out, rows, reps=reps)
        else:
            x = nc.dram_tensor("x", [rows, D], F32, kind="ExternalInput").ap()
            w = nc.dram_tensor("W", [CROSS, D], F32, kind="ExternalInput").ap()
            b = nc.dram_tensor("b", [CROSS, D], F32, kind="ExternalInput").ap()
            out = nc.dram_tensor("out", [rows, D], F32,
                                 kind="ExternalOutput").ap()
            with tile.TileContext(nc) as tc:
                build_body_general(tc, x, w, b, out, rows)
        nc.compile()
        _CACHE[key] = nc
    return _CACHE[key]


def pack_wt(W):
    """Host packing of W [3, D] -> Wt [128, 4*NB] with
    Wt[p, 4*b + i] = W[i, 128*b + p] (i < 3; lane 3 zero)."""
    Wt = np.zeros((P, 4 * NB), np.float32)
    Wt.reshape(P, NB, 4)[:, :, :CROSS] = W.reshape(CROSS, NB, P).transpose(2, 1, 0)
    return Wt


def run(x, W, b, trace=False, force_general=False):
    x = np.ascontiguousarray(np.asarray(x, dtype=np.float32))
    W = np.ascontiguousarray(np.asarray(W, dtype=np.float32))
    b = np.ascontiguousarray(np.asarray(b, dtype=np.float32))
    rows = x.shape[0] // N_CORES
    zero_b = (not force_general) and not b.any()
    nc = get_nc(rows, zero_b)
    if zero_b:
        wt = pack_wt(W)
        ident = np.eye(P, dtype=np.float32)
        in_maps = [
            {
                "x": x[i * rows : (i + 1) * rows].reshape(rows // 2, 2 * D),
                "Wt": wt,
                "I": ident,
            }
            for i in range(N_CORES)
        ]
    else:
        in_maps = [
            {"x": x[i * rows : (i + 1) * rows], "W": W, "b": b}
            for i in range(N_CORES)
        ]
    try:
        res = run_bass_kernel_spmd(
            nc, in_maps, core_ids=list(range(N_CORES)), trace=trace
        )
    except ModuleNotFoundError:
        # BASS_TRACE in the environment routes through an NTFF profile hook
        # that is absent in some containers; fall back to an untraced run.
        import os

        os.environ["BASS_NEVER_TRACE"] = "1"
        res = run_bass_kernel_spmd(
            nc, in_maps, core_ids=list(range(N_CORES)), trace=False
        )
    out = np.concatenate(
        [r["out"].reshape(rows, D) for r in res.results], axis=0
    )
    return out, res


def kernel(x, W, b):
    out, _ = run(x, W, b)
    return out


# revision 6
# speedup vs baseline: 1.2074x; 1.2074x over previous
"""CrossNetwork (DCN) forward on 8 TRN2 NeuronCores.

Reference computation (per cross layer i, x0 = input):
    s_i = xl . w_i            (per-row scalar)
    xl  = x0 * s_i + b_i + xl

Algebraic collapse: xl_i = alpha_i * x0 + c_i with per-row scalar alpha_i
and a row-constant vector c_i = sum_{j<i} b_j. Hence:
    u_i       = x0 . w_i                      (3 dots per row, all vs x0)
    alpha_0   = 1,  alpha_{i+1} = alpha_i * (1 + u_i) + (c_i . w_i)
    out       = alpha_3 * x0 + c_3
One read of x, one write of out -> memory roofline (32 MiB HBM per core).

Sharding: pure data parallel over the batch dim, weights replicated.

zero-b fast path (the reference always passes b = 0): the three dots are
computed on the otherwise-idle PE array.  Each 128-row sub-tile is
transposed on-chip (PE transpose -> PSUM -> DVE/ACT copy to SBUF), then a
second accumulating matmul contracts over D with the host-packed W^T as
the moving operand, producing u = [128 rows, 4] directly in PSUM with
rows on partitions.  alpha3 = (1+u0)(1+u1)(1+u2) is three tiny ACT ops;
the final scale out = alpha3 * x is split DVE/ACT.  fp32 end to end.
Elementwise engines only do: PSUM->SBUF copy (1 pass) + scale (1 pass),
so every engine stays well under the ~94 us DMA roofline, with DMA in
2 MiB transfers (loads on SP HWDGE, stores on Pool SWDGE).

A general-b variant (full constants) is kept for the b != 0 case.
"""

import contextlib

import numpy as np

import concourse.bacc as bacc
import concourse.mybir as mybir
import concourse.tile as tile
from concourse.bass_utils import run_bass_kernel_spmd

N_CORES = 8
B, D, CROSS = 16384, 2048, 3
P = 128
NB = D // P  # 16 column blocks per sub-tile
F32 = mybir.dt.float32


def build_body_zero_b(tc, x_ap, wt_ap, ident_ap, out_ap, rows, reps=1,
                      v2=False):
    """b == 0 specialization: out = alpha3 * x, dots on the PE array.

    x_ap/out_ap are [rows//2, 2*D] views of the row-major [rows, D] data:
    big tile t = 128 view-rows = 256 data rows; sub-tile j in {0,1} is
    x[:, j*D:(j+1)*D] = data rows 256*t + 2*p + j.

    v2: deeper x/y buffering and stores alternating between the ACT HWDGE
    ring and Pool SWDGE (halves SWDGE descriptor-ring traffic, which
    contends with SDMA engines 7/15 on the partition-0..31 AXI ports).

    reps > 1 repeats the main loop in-NEFF (benchmarking only).
    """
    nc = tc.nc
    nt = rows // 256  # big tiles per pass
    Act = mybir.ActivationFunctionType

    with contextlib.ExitStack() as ctx:
        const = ctx.enter_context(tc.tile_pool(name="const", bufs=1))
        xpool = ctx.enter_context(tc.tile_pool(name="x", bufs=5 if v2 else 4))
        ypool = ctx.enter_context(tc.tile_pool(name="y", bufs=4 if v2 else 3))
        xtpool = ctx.enter_context(tc.tile_pool(name="xT", bufs=3))
        ptpool = ctx.enter_context(
            tc.tile_pool(name="pT", bufs=6, space="PSUM")
        )
        utpool = ctx.enter_context(
            tc.tile_pool(name="u", bufs=2, space="PSUM")
        )
        apool = ctx.enter_context(tc.tile_pool(name="a", bufs=12))

        ident = const.tile([P, P], F32, tag="I")
        nc.scalar.dma_start(out=ident[:], in_=ident_ap[:, :])
        wt = const.tile([P, 4 * NB], F32, tag="Wt")
        nc.scalar.dma_start(out=wt[:], in_=wt_ap[:, :])

        def stage_b(st):
            """Dots + recurrence + scale + (on j==1) store for one sub-tile."""
            xs, xts, yt, t, j, k = st
            ut = utpool.tile([P, 4], F32, tag="u")
            for b in range(NB):
                nc.tensor.matmul(
                    out=ut[:],
                    lhsT=xts[:, b * P : (b + 1) * P],
                    rhs=wt[:, b * 4 : (b + 1) * 4],
                    start=(b == 0),
                    stop=(b == NB - 1),
                )
            # alpha3 = (1+u0)(1+u1)(1+u2): ACT tiny ops on [P, 1]
            t1 = apool.tile([P, 1], F32, tag="t1")
            nc.scalar.add(t1[:], ut[:, 0:1], 1.0)
            a2 = apool.tile([P, 1], F32, tag="a2")
            nc.scalar.activation(a2[:], ut[:, 1:2], Act.Identity,
                                 bias=t1[:], scale=t1[:])
            a3 = apool.tile([P, 1], F32, tag="a3")
            nc.scalar.activation(a3[:], ut[:, 2:3], Act.Identity,
                                 bias=a2[:], scale=a2[:])
            # out = alpha3 * x0
            ys = yt[:, j * D : (j + 1) * D]
            if k % 2 == 0:
                nc.vector.tensor_scalar_mul(ys, xs, a3[:])
            else:
                nc.scalar.activation(ys, xs, Act.Copy, scale=a3[:])
            if j == 1:
                t_ = t % nt
                st_eng = nc.scalar if (v2 and t % 2) else nc.gpsimd
                st_eng.dma_start(
                    out=out_ap[t_ * P : (t_ + 1) * P, :], in_=yt[:]
                )

        prev = None
        xt = yt = None
        for k in range(nt * reps * 2):
            t, j = k // 2, k % 2
            if j == 0:
                t_ = t % nt
                xt = xpool.tile([P, 2 * D], F32, tag="x")
                nc.sync.dma_start(
                    out=xt[:], in_=x_ap[t_ * P : (t_ + 1) * P, :]
                )
                yt = ypool.tile([P, 2 * D], F32, tag="y")
            xs = xt[:, j * D : (j + 1) * D]

            # stage A: 16 PE transposes into 4 PSUM bank tiles, then 4
            # PSUM->SBUF copies (DVE/ACT split)
            pts = []
            for c in range(4):
                pt = ptpool.tile([P, 512], F32, tag="pT")
                for q in range(4):
                    b = c * 4 + q
                    nc.tensor.transpose(
                        pt[:, q * P : (q + 1) * P],
                        xs[:, b * P : (b + 1) * P],
                        ident[:],
                    )
                pts.append(pt)
            xts = xtpool.tile([P, D], F32, tag="xT")
            for c in range(4):
                dst = xts[:, c * 512 : (c + 1) * 512]
                if c < 2:
                    nc.vector.tensor_copy(dst, pts[c][:])
                else:
                    nc.scalar.activation(dst, pts[c][:], Act.Copy)

            # stage B of the previous sub-tile (keeps PE pipelined: its
            # u-matmuls wait on copies that overlap this sub-tile's
            # transposes)
            if prev is not None:
                stage_b(prev)
            prev = (xs, xts, yt, t, j, k)
        stage_b(prev)


def build_body_general(tc, x_ap, w_ap, b_ap, out_ap, rows):
    """General-b path: full constants, final = ACT scale + Pool bias-add."""
    nc = tc.nc
    nt = rows // P
    Al = mybir.AluOpType
    Act = mybir.ActivationFunctionType

    with contextlib.ExitStack() as ctx:
        const = ctx.enter_context(tc.tile_pool(name="const", bufs=1))
        xpool = ctx.enter_context(tc.tile_pool(name="x", bufs=4))
        ypool = ctx.enter_context(tc.tile_pool(name="y", bufs=4))
        spool = ctx.enter_context(tc.tile_pool(name="scr", bufs=3))
        upool = ctx.enter_context(tc.tile_pool(name="u", bufs=16))

        # Load each tiny w_i / b_i row to partition 0, then replicate across
        # all 128 partitions on-chip (gpsimd partition_broadcast). The custom
        # op requires its input AP to start at partition 0, hence one [1, D]
        # tile per row. All row tiles are transient (pre pool).
        with tc.tile_pool(name="pre", bufs=1) as pre:
            wrow = []
            brow = []
            for i in range(CROSS):
                wr = pre.tile([1, D], F32, tag=f"wr{i}")
                nc.sync.dma_start(out=wr[:], in_=w_ap[i : i + 1, :])
                wrow.append(wr)
                br = pre.tile([1, D], F32, tag=f"br{i}")
                nc.sync.dma_start(out=br[:], in_=b_ap[i : i + 1, :])
                brow.append(br)

            wbc = []
            for i in range(CROSS):
                wt = const.tile([P, D], F32, tag=f"w{i}")
                nc.gpsimd.partition_broadcast(wt[:], wrow[i][:])
                wbc.append(wt)

            # row constants on [1, D]: c2 = b0 + b1, c3 = c2 + b2
            c2row = pre.tile([1, D], F32, tag="c2r")
            nc.vector.tensor_add(c2row[:], brow[0][:], brow[1][:])
            c3row = pre.tile([1, D], F32, tag="c3r")
            nc.vector.tensor_add(c3row[:], c2row[:], brow[2][:])
            c3bc = const.tile([P, D], F32, tag="c3")
            nc.gpsimd.partition_broadcast(c3bc[:], c3row[:])

            # k1 = b0 . w1, k2 = c2 . w2 (scalars), then replicate to [P, 1]
            k1row = pre.tile([1, 1], F32, tag="k1r")
            scr_k1 = pre.tile([1, D], F32, tag="scrr")
            nc.vector.scalar_tensor_tensor(
                out=scr_k1[:], in0=brow[0][:], scalar=0.0, in1=wrow[1][:],
                op0=Al.bypass, op1=Al.mult, accum_out=k1row[:],
            )
            k2row = pre.tile([1, 1], F32, tag="k2r")
            scr_k2 = pre.tile([1, D], F32, tag="scrr2")
            nc.vector.scalar_tensor_tensor(
                out=scr_k2[:], in0=c2row[:], scalar=0.0, in1=wrow[2][:],
                op0=Al.bypass, op1=Al.mult, accum_out=k2row[:],
            )
            k1bc = const.tile([P, 1], F32, tag="k1")
            nc.gpsimd.partition_broadcast(k1bc[:], k1row[:])
            k2bc = const.tile([P, 1], F32, tag="k2")
            nc.gpsimd.partition_broadcast(k2bc[:], k2row[:])

        for t in range(nt):
            xt = xpool.tile([P, D], F32, tag="x")
            nc.sync.dma_start(out=xt[:], in_=x_ap[t * P : (t + 1) * P, :])

            us = []
            for i in range(CROSS):
                u = upool.tile([P, 1], F32, tag=f"u{i}")
                scr = spool.tile([P, D], F32, tag="scr")
                nc.vector.scalar_tensor_tensor(
                    out=scr[:], in0=xt[:], scalar=0.0, in1=wbc[i][:],
                    op0=Al.bypass, op1=Al.mult, accum_out=u[:],
                )
                us.append(u)

            # alpha recurrence on ACT: a3 = ((1+u0)(1+u1) + k1)(1+u2) + k2
            t1 = upool.tile([P, 1], F32, tag="t1")
            nc.scalar.add(t1[:], us[0][:], 1.0)
            t2 = upool.tile([P, 1], F32, tag="t2")
            nc.scalar.add(t2[:], us[1][:], 1.0)
            a2 = upool.tile([P, 1], F32, tag="a2")
            nc.scalar.activation(a2[:], t2[:], Act.Identity, bias=k1bc[:], scale=t1[:])
            t3 = upool.tile([P, 1], F32, tag="t3")
            nc.scalar.add(t3[:], us[2][:], 1.0)
            a3 = upool.tile([P, 1], F32, tag="a3")
            nc.scalar.activation(a3[:], t3[:], Act.Identity, bias=k2bc[:], scale=a2[:])

            # out = alpha3 * x0 + c3: scale on ACT, bias-add in place on Pool
            yt = ypool.tile([P, D], F32, tag="y")
            nc.scalar.activation(yt[:], xt[:], Act.Copy, scale=a3[:])
            nc.gpsimd.tensor_tensor(out=yt[:], in0=yt[:], in1=c3bc[:], op=Al.add)
            nc.sync.dma_start(out=out_ap[t * P : (t + 1) * P, :], in_=yt[:])


_CACHE = {}


def get_nc(rows, zero_b=False, reps=1, v2=False):
    key = (rows, zero_b, reps, v2)
    if key not in _CACHE:
        nc = bacc.Bacc(
            "TRN2",
            target_bir_lowering=False,
            debug=False,
            enable_asserts=False,
            num_devices=N_CORES,
        )
        if zero_b:
            x = nc.dram_tensor("x", [rows // 2, 2 * D], F32,
                               kind="ExternalInput").ap()
            wt = nc.dram_tensor("Wt", [P, 4 * NB], F32,
                                kind="ExternalInput").ap()
            ident = nc.dram_tensor("I", [P, P], F32, kind="ExternalInput").ap()
            out = nc.dram_tensor("out", [rows // 2, 2 * D], F32,
                                 kind="ExternalOutput").ap()
            with tile.TileContext(nc) as tc:
                build_body_zero_b(tc, x, wt, ident, out, rows, reps=reps,
                                  v2=v2)
        else:
            x = nc.dram_tensor("x", [rows, D], F32, kind="ExternalInput").ap()
            w = nc.dram_tensor("W", [CROSS, D], F32, kind="ExternalInput").ap()
            b = nc.dram_tensor("b", [CROSS, D], F32, kind="ExternalInput").ap()
            out = nc.dram_tensor("out", [rows, D], F32,
                                 kind="ExternalOutput").ap()
            with tile.TileContext(nc) as tc:
                build_body_general(tc, x, w, b, out, rows)
        nc.compile()
        _CACHE[key] = nc
    return _CACHE[key]


def pack_wt(W):
    """Host packing of W [3, D] -> Wt [128, 4*NB] with
    Wt[p, 4*b + i] = W[i, 128*b + p] (i < 3; lane 3 zero)."""
    Wt = np.zeros((P, 4 * NB), np.float32)
    Wt.reshape(P, NB, 4)[:, :, :CROSS] = W.reshape(CROSS, NB, P).transpose(2, 1, 0)
    return Wt


def run(x, W, b, trace=False, force_general=False):
    x = np.ascontiguousarray(np.asarray(x, dtype=np.float32))
    W = np.ascontiguousarray(np.asarray(W, dtype=np.float32))
    b = np.ascontiguousarray(np.asarray(b, dtype=np.float32))
    rows = x.shape[0] // N_CORES
    zero_b = (not force_general) and not b.any()
    nc = get_nc(rows, zero_b)
    if zero_b:
        wt = pack_wt(W)
        ident = np.eye(P, dtype=np.float32)
        in_maps = [
            {
                "x": x[i * rows : (i + 1) * rows].reshape(rows // 2, 2 * D),
                "Wt": wt,
                "I": ident,
            }
            for i in range(N_CORES)
        ]
    else:
        in_maps = [
            {"x": x[i * rows : (i + 1) * rows], "W": W, "b": b}
            for i in range(N_CORES)
        ]
    try:
        res = run_bass_kernel_spmd(
            nc, in_maps, core_ids=list(range(N_CORES)), trace=trace
        )
    except ModuleNotFoundError:
        # BASS_TRACE in the environment routes through an NTFF profile hook
        # that is absent in some containers; fall back to an untraced run.
        import os

        os.environ["BASS_NEVER_TRACE"] = "1"
        res = run_bass_kernel_spmd(
            nc, in_maps, core_ids=list(range(N_CORES)), trace=False
        )
    out = np.concatenate(
        [r["out"].reshape(rows, D) for r in res.results], axis=0
    )
    return out, res


def kernel(x, W, b):
    out, _ = run(x, W, b)
    return out
